# revision 89
# baseline (speedup 1.0000x reference)
"""Trainium2 Bass kernel for a sparse-attention (sliding-window) transformer block.

Reference computation (per batch b, token t):
    x = x + attn(rmsnorm(x, ln1_w));  attn = (windowed_softmax(qk)·v * sigmoid(gate)) @ out_w.T
    out = x + swiglu_ffn(rmsnorm(x, ln2_w))

Sharding: token-parallel across 8 cores (B=2 x 4 chunks of 512 tokens).  Each
core receives its 512 tokens plus the previous 256 tokens (sliding-window halo,
zeros for the first chunk) and recomputes K/V on the halo -> no collectives.

v3 versus v2 (~338us TimelineSim): fp8-e4m3 DoubleRow matmuls for the
attention-side GEMMs (q/k/v/gate projections, AV, out-projection) at 2x PE
throughput; scores (contraction 64) and the whole FFN stay bf16 -- fp8 noise
on the FFN branch alone costs ~2.4e-2 relative error (75% of output norm),
while the attention side costs ~8e-3 (CPU-sim measured).  All quantization
scales are powers of two computed from the weights at runtime and folded
into existing instructions:
  - balanced per-channel q/k scales s_d, (c/s_d) cancel in q.k up to the
    global c, which folds into the exp() scale argument,
  - v dequant folds into the PSUM->SBUF copy scale; the fp8 ones-column
    (value SV8) makes the softmax divide remove v's fp8 scale for free,
  - gate dequant folds into the sigmoid scale,
  - the out-projection scale s_o stays ON the residual stream: xf is
    pre-scaled by s_o on the host, rms2's eps becomes eps*s_o^2 (rmsnorm is
    scale-invariant), and the final identity-residual matmul uses a 1/s_o
    diagonal so the output lands unscaled.
"""

import os
import sys

import numpy as np

if "/opt/trn_rl_repo" not in sys.path:
    sys.path.insert(0, "/opt/trn_rl_repo")

# ---- problem constants (hardcoded; kernel.py must be self-contained) ----
D = 1024          # d_model
NH = 16           # heads
DH = 64           # head dim
DFF = 4096        # ffn hidden
WIN = 256         # sliding window
B, T = 2, 2048
EPS = 1e-6

NCORES = 8
CHUNK = 512       # own tokens per core
HALO = 256        # preceding-token halo
LT = CHUNK + HALO  # 768 local tokens (halo first)
P = 128
ND = D // P       # 8
NF = DFF // P     # 32

SCALE = DH ** -0.5
SA = 16.0         # fp8 h1 activation scale
SV8 = 16.0        # fp8 v scale
TGT = 112.0       # fp8 weight absmax target (e4m3 max 240)
EXP_SHIFT = -4.0  # constant shift inside exp; softmax-invariant

_CACHE = {}


# --------------------------------------------------------------------------
# program builder
# --------------------------------------------------------------------------

def build_program(sc):
    """sc: dict of scalar fold constants (EXPS, SIG, KV, EPS2, from weights)."""
    import concourse.bacc as bacc
    import concourse.tile as tile
    from concourse import mybir

    f32 = mybir.dt.float32
    bf = mybir.dt.bfloat16
    f8 = mybir.dt.float8e4

    nc = bacc.Bacc("TRN2", target_bir_lowering=False, debug=False,
                   num_devices=NCORES)

    io = {}
    # activations (xb = fp8 of SA*x, raw; rmsnorm folds into PSUM copies)
    io["xb"] = nc.dram_tensor("xb", [P, ND, LT], f8, kind="ExternalInput").ap()
    io["xf"] = nc.dram_tensor("xf", [P, ND, CHUNK], f32,
                              kind="ExternalInput").ap()
    # attention weights (fp8), pre-tiled [partition, tiles..., free]
    io["wq"] = nc.dram_tensor("wq", [P, ND, ND, P], f8, kind="ExternalInput").ap()
    io["wk"] = nc.dram_tensor("wk", [P, ND, ND, P], f8, kind="ExternalInput").ap()
    io["wv"] = nc.dram_tensor("wv", [P, 4, ND, 256], f8, kind="ExternalInput").ap()
    io["wgt"] = nc.dram_tensor("wgt", [P, ND, ND, P], f8, kind="ExternalInput").ap()
    io["wo_a"] = nc.dram_tensor("wo_a", [P, ND, ND, P], f8, kind="ExternalInput").ap()
    # ffn weights (bf16)
    io["wg"] = nc.dram_tensor("wg", [P, NF, ND, P], bf, kind="ExternalInput").ap()
    io["wu"] = nc.dram_tensor("wu", [P, NF, ND, P], bf, kind="ExternalInput").ap()
    io["wo"] = nc.dram_tensor("wo", [P, NF, D], bf, kind="ExternalInput").ap()
    # triangle bias consts: the sliding-window mask folds into the score
    # PSUM accumulation as one identity-stationary matmul per k-tile,
    # pushing masked logits to -1e10 so exp flushes them to fp8 zero
    io["trimask"] = nc.dram_tensor("trimask", [P, 2, 4, 256], bf,
                                   kind="ExternalInput").ap()
    io["iden"] = nc.dram_tensor("iden", [P, P], bf, kind="ExternalInput").ap()
    io["consts"] = nc.dram_tensor("consts", [1, 2, P], bf,
                                  kind="ExternalInput").ap()
    io["outT"] = nc.dram_tensor("outT", [D, CHUNK], bf,
                                kind="ExternalOutput").ap()

    if os.environ.get("BASS_TAPS") == "1":
        for nm, shape, dt in [
                ("dbg_q", [P, ND, CHUNK], bf),
                ("dbg_k", [P, ND, LT], bf),
                ("dbg_v", [P, (LT // P) * NH * DH], f8),
                ("dbg_gate", [P, ND, CHUNK], bf),
                ("dbg_at8", [P, ND, CHUNK], f8),
                ("dbg_x2", [P, ND, CHUNK], f32),
                ("dbg_h2", [P, ND, CHUNK], bf)]:
            io[nm] = nc.dram_tensor(nm, shape, dt, kind="ExternalOutput").ap()

    with tile.TileContext(nc) as tc:
        _emit(tc, io, sc)

    nc.compile()
    return nc


def _emit(tc, io, sc):
    from contextlib import ExitStack

    from concourse import mybir

    nc = tc.nc
    f32 = mybir.dt.float32
    bf = mybir.dt.bfloat16
    f8 = mybir.dt.float8e4
    AF = mybir.ActivationFunctionType
    DR = mybir.MatmulPerfMode.DoubleRow

    # round-robin DMA issue across the three queues; eng= overrides
    _q = [0]

    def dma(out, in_, eng=None):
        if eng is None:
            eng = (nc.sync, nc.scalar, nc.gpsimd)[_q[0] % 3]
            _q[0] += 1
        eng.dma_start(out=out, in_=in_)

    def tap(name, src_ap):
        if name in io:
            nc.sync.dma_start(out=io[name], in_=src_ap)

    FPRE = 4  # ffn fo-tiles prefetched during attention

    with ExitStack() as ctx:
        ctx.enter_context(nc.allow_low_precision(
            reason="fp8/bf16 matmul inputs; all accumulation stays fp32 PSUM"))
        glob = ctx.enter_context(tc.tile_pool(name="glob", bufs=1))

        # ---- phase 0: global constants + input prefetch ------------------
        consts = glob.tile([1, 2, P], bf, name="consts")
        dma(consts[:], io["consts"], eng=nc.sync)
        xTp = ctx.enter_context(tc.tile_pool(name="xTp", bufs=1))
        xT = xTp.tile([P, ND, LT], f8, name="xT")
        xb_engs = (nc.sync, nc.scalar, nc.gpsimd, nc.sync)
        for i, a in enumerate(range(0, ND, 2)):
            dma(xT[:, a:a + 2, :], io["xb"][:, a:a + 2, :], eng=xb_engs[i])
        maskT = glob.tile([P, 2, 4, 256], bf, name="maskT")

        onescol_f = glob.tile([P, 1], f32)
        nc.vector.memset(onescol_f, 1.0)
        onescol = glob.tile([P, 1], bf)
        nc.vector.tensor_copy(onescol[:], onescol_f[:])
        epsb = glob.tile([P, 1], f32)
        nc.vector.memset(epsb, EPS)
        eps2b = glob.tile([P, 1], f32)
        nc.vector.memset(eps2b, sc["EPS2"])
        shiftb = glob.tile([P, 1], f32)
        nc.vector.memset(shiftb, EXP_SHIFT)

        # prewarm the Sqrt table (rms1 critical path); the other tables are
        # warmed right after the rms sqrt so they never delay it
        warm = glob.tile([1, 1], f32, name="warm")
        nc.scalar.activation(warm[:], epsb[0:1, :], AF.Sqrt)

        # residual fp32 s_o*x (host-prescaled; loaded at out-proj time)
        xF = glob.tile([P, ND, CHUNK], f32, name="xF")

        # identity/s_o stationary for PE-side residual accumulation
        iden = glob.tile([P, P], bf, name="iden")
        dma(iden[:], io["iden"], eng=nc.sync)
        # trimask is needed only once attention starts; keep it off the Act
        # queue (its dma_start issue would delay the rms sqrt)
        dma(maskT[:], io["trimask"], eng=nc.sync)

        # ffn weights: persistent pools; wg/wu prefetch FPRE tiles early,
        # wo streams within the wo pass (bufs ahead) to avoid recycle stalls
        wgp = ctx.enter_context(tc.tile_pool(name="wgp", bufs=FPRE + 1))
        wup = ctx.enter_context(tc.tile_pool(name="wup", bufs=FPRE + 1))
        wop = ctx.enter_context(tc.tile_pool(name="wop", bufs=4))
        wg_tiles, wu_tiles, wo_tiles = {}, {}, {}

        # ffn weight streaming stays off the Act queue (exp/silu live there)
        def gu_fetch(fo):
            wg_tiles[fo] = wgp.tile([P, ND, P], bf, name="wgf")
            dma(wg_tiles[fo][:], io["wg"][:, fo], eng=nc.sync)
            wu_tiles[fo] = wup.tile([P, ND, P], bf, name="wuf")
            dma(wu_tiles[fo][:], io["wu"][:, fo], eng=nc.gpsimd)

        def wo_fetch(fo):
            wo_tiles[fo] = wop.tile([P, D], bf, name="wof")
            dma(wo_tiles[fo][:], io["wo"][:, fo],
                eng=(nc.sync, nc.gpsimd)[fo % 2])

        # ---- phase 1: rmsnorm1 over all LT tokens ------------------------
        with ExitStack() as actx:
            qT = actx.enter_context(tc.tile_pool(name="qTp", bufs=1)).tile(
                [P, ND, CHUNK], bf, name="qT")
            kT = actx.enter_context(tc.tile_pool(name="kTp", bufs=1)).tile(
                [P, ND, LT], bf, name="kT")
            vaug = actx.enter_context(tc.tile_pool(name="vaugp", bufs=1)).tile(
                [P, LT // P, NH, DH], f8, name="vaug")
            gateT = actx.enter_context(tc.tile_pool(name="gateTp", bufs=1)).tile(
                [P, ND, CHUNK], bf, name="gateT")
            at8T = actx.enter_context(tc.tile_pool(name="at8Tp", bufs=1)).tile(
                [P, ND, CHUNK], f8, name="at8T")

            # projection weights: resident for phases 1-2 only
            wvt = actx.enter_context(tc.tile_pool(name="wvtp", bufs=1)).tile(
                [P, 4, ND, 256], f8, name="wvt")
            deqp = actx.enter_context(tc.tile_pool(name="deqp", bufs=1))
            rbc = deqp.tile([P, 2, LT], bf, name="rbc")
            rvk = deqp.tile([P, 6], f32, name="rvk")
            projw_sc = ExitStack()
            projw = projw_sc.enter_context(tc.tile_pool(name="projw", bufs=1))
            wqt = projw.tile([P, ND, ND, P], f8, name="wqt")
            wkt = projw.tile([P, ND, ND, P], f8, name="wkt")
            wgtt = projw.tile([P, ND, ND, P], f8, name="wgtt")
            # first-use order: q, k, v, gate
            for dst, src in ((wqt, "wq"), (wkt, "wk")):
                dma(dst[:, 0:4], io[src][:, 0:4], eng=nc.sync)
                dma(dst[:, 4:8], io[src][:, 4:8], eng=nc.gpsimd)
            dma(wvt[:, 0:2], io["wv"][:, 0:2], eng=nc.sync)
            dma(wvt[:, 2:4], io["wv"][:, 2:4], eng=nc.gpsimd)
            dma(wgtt[:, 0:4], io["wgt"][:, 0:4], eng=nc.sync)
            dma(wgtt[:, 4:8], io["wgt"][:, 4:8], eng=nc.gpsimd)

            with ExitStack() as pctx:
                sqp = pctx.enter_context(tc.tile_pool(name="sqp", bufs=2))
                msp = pctx.enter_context(
                    tc.tile_pool(name="msp", bufs=2, space="PSUM"))
                rbp = pctx.enter_context(
                    tc.tile_pool(name="rbp", bufs=2, space="PSUM"))
                rowp = pctx.enter_context(tc.tile_pool(name="rowp", bufs=2))

                msg = [msp.tile([1, 384], f32, name=f"ms{g}", tag=f"ms{g}")
                       for g in range(2)]
                for a in range(ND):
                    sq = sqp.tile([P, LT], bf, name="sq")
                    # x is fp8(SA*x); squares split across DVE/Act/Pool
                    if a % 3 == 2:
                        nc.scalar.activation(sq[:], xT[:, a, :], AF.Square)
                    else:
                        eng = nc.vector if a % 3 == 0 else nc.gpsimd
                        eng.tensor_mul(sq[:], xT[:, a, :], xT[:, a, :])
                    for g in range(2):
                        sl = slice(g * 384, (g + 1) * 384)
                        nc.tensor.matmul(msg[g][:], onescol[:], sq[:, sl],
                                         start=(a == 0), stop=(a == ND - 1))
                sr = rowp.tile([1, LT], f32, name="sr")
                for g in range(2):
                    nc.scalar.activation(sr[:, g * 384:(g + 1) * 384],
                                         msg[g][:], AF.Sqrt,
                                         bias=epsb[0:1, :],
                                         scale=1.0 / (D * SA * SA))
                # warm the remaining Act tables now (Act idles during the
                # projection matmuls; swaps would otherwise hit attention)
                for wf in (AF.Exp, AF.Sigmoid, AF.Square, AF.Silu):
                    nc.scalar.activation(warm[:], epsb[0:1, :], wf)
                rro = rowp.tile([1, LT], bf, name="rro")
                nc.vector.reciprocal(rro[:], sr[:])
                # rbc duplicated x2 (pair-merged dequant muls); rmsnorm is
                # deferred through the projections: rbc = 1/rms per token
                for g in range(2):
                    sl = slice(g * 384, (g + 1) * 384)
                    rb = rbp.tile([P, 384], f32, name="rb")
                    nc.tensor.matmul(rb[:], consts[0:1, 0, :], rro[:, sl],
                                     start=True, stop=True)
                    nc.vector.tensor_copy(rbc[:, 0, sl], rb[:])
                    nc.scalar.copy(rbc[:, 1, sl], rb[:])
                # per-token-tile 1/rms columns for the v-projection copies
                # (gpsimd DMA can cast bf16->f32; KV folds into the copies'
                # second tensor_scalar operand, so no extra DVE op here)
                for tt in range(6):
                    nc.gpsimd.dma_start(out=rvk[:, tt:tt + 1],
                                        in_=rro[0:1, tt * P:(tt + 1) * P])

            # fp8 all-SV8 stationary for the denominator-block matmuls (the
            # softmax divide dequantizes v's fp8 scale); 64 replicated rows
            # so the whole reciprocal is one DVE op and no broadcast needed
            onesbf = glob.tile([P, 2, DH], f32)
            nc.vector.memset(onesbf, SV8)
            onesb = glob.tile([P, 2, DH], f8)
            nc.vector.tensor_copy(onesb[:], onesbf[:])


            # ---- phase 2: q/k/v/gate projections (fp8 DoubleRow) ---------
            # projections consume raw fp8 x; the deferred 1/rms rides the
            # PSUM->SBUF dequant ops (DVE/Pool mults, Act per-partition scale)
            with ExitStack() as pctx:
                pj = pctx.enter_context(
                    tc.tile_pool(name="pj", bufs=4, space="PSUM"))
                sgp2 = pctx.enter_context(tc.tile_pool(name="sgp2", bufs=2))

                # q^T [D, CHUNK] (own tokens only); holds SA*s_d*q_d
                for po in range(0, ND, 2):
                    ps = pj.tile([P, 2, CHUNK], f32, name="ps", tag="pj")
                    for w in range(2):
                        for pi in range(0, ND, 2):
                            nc.tensor.matmul(ps[:, w, :],
                                             wqt[:, po + w, pi:pi + 2, :],
                                             xT[:, pi:pi + 2, HALO:LT],
                                             start=(pi == 0),
                                             stop=(pi == ND - 2),
                                             perf_mode=DR)
                    nc.vector.tensor_mul(qT[:, po:po + 2, :], ps[:],
                                         rbc[:, :, HALO:LT])

                # k^T [D, LT]; holds SA*(c/s_d)*k_d.  PSUM slabs are padded
                # to 2KB so each accumulation group stays within one bank
                for po in range(ND):
                    ps = pj.tile([P, 2, CHUNK], f32, name="psk", tag="pj")
                    for g in range(2):
                        sl = slice(g * 384, (g + 1) * 384)
                        for pi in range(0, ND, 2):
                            nc.tensor.matmul(ps[:, g, 0:384],
                                             wkt[:, po, pi:pi + 2, :],
                                             xT[:, pi:pi + 2, sl],
                                             start=(pi == 0),
                                             stop=(pi == ND - 2),
                                             perf_mode=DR)
                    nc.vector.tensor_mul(
                        kT[:, po, :], ps[:, :, 0:384],
                        rbc[:, 0, :].rearrange("p (g t) -> p g t", g=2))

                # v [LT, D], token-major (tokens on PSUM partitions): ng-pair
                # PSUM tiles (heads contiguous in vaug) -> 12 Act copies with
                # the per-partition KV/rms dequant scale
                for tt in range(6):
                    for ng in range(0, 4, 2):
                        ps = pj.tile([P, 2, 256], f32, name="psv", tag="pj")
                        for w in range(2):
                            for pi in range(0, ND, 2):
                                nc.tensor.matmul(
                                    ps[:, w, :],
                                    xT[:, pi:pi + 2, tt * P:(tt + 1) * P],
                                    wvt[:, ng + w, pi:pi + 2, :],
                                    start=(pi == 0), stop=(pi == ND - 2),
                                    perf_mode=DR)
                        # DVE (per-partition scalar) keeps the Act queue
                        # clear so the first exps start right after phase 2
                        nc.vector.tensor_scalar(
                            vaug[:, tt, 4 * ng:4 * (ng + 2), :],
                            ps[:].rearrange("p two (h d) -> p (two h) d",
                                            d=DH),
                            rvk[:, tt:tt + 1], sc["KV"],
                            op0=mybir.AluOpType.mult,
                            op1=mybir.AluOpType.mult)

                # gate^T = sigmoid(r/(SA*s_g) * psum)^T [D, CHUNK]
                for po in range(0, ND, 2):
                    ps = pj.tile([P, 2, CHUNK], f32, name="ps", tag="pj")
                    for w in range(2):
                        for pi in range(0, ND, 2):
                            nc.tensor.matmul(ps[:, w, :],
                                             wgtt[:, po + w, pi:pi + 2, :],
                                             xT[:, pi:pi + 2, HALO:LT],
                                             start=(pi == 0),
                                             stop=(pi == ND - 2),
                                             perf_mode=DR)
                    gp = sgp2.tile([P, 2, CHUNK], bf, name="gp")
                    nc.vector.scalar_tensor_tensor(
                        gp[:], ps[:], sc["SIG"], rbc[:, :, HALO:LT],
                        op0=mybir.AluOpType.mult, op1=mybir.AluOpType.mult)
                    nc.scalar.activation(gateT[:, po:po + 2, :], gp[:],
                                         AF.Sigmoid)

            tap("dbg_q", qT[:])
            tap("dbg_k", kT[:])
            tap("dbg_v", vaug[:].rearrange("p a h d -> p (a h d)"))
            tap("dbg_gate", gateT[:])

            projw_sc.close()  # free the projection weights' SBUF

            # ---- phase 3+4: attention, depth-2 software pipeline ----------
            # per pair: PE scores (8 matmuls) -> Act exp (2 ops) -> mask mult
            # split DVE/Pool -> PE AV + denominators (DoubleRow) -> recip ->
            # broadcast -> one fused divide + one gate mult per pair.
            # consume(p-2) runs while Act/DVE chew on pairs p-1/p.
            rro2 = glob.tile([1, CHUNK], bf, name="rro2")
            with ExitStack() as pctx:
                pop = pctx.enter_context(
                    tc.tile_pool(name="pop", bufs=1, space="PSUM"))
                wsp4 = pctx.enter_context(tc.tile_pool(name="wsp4", bufs=8))
                sqp5 = pctx.enter_context(tc.tile_pool(name="sqp5", bufs=2))
                r2p = pctx.enter_context(tc.tile_pool(name="r2p", bufs=1))
                att_sc = ExitStack()
                avp = att_sc.enter_context(
                    tc.tile_pool(name="avp", bufs=1, space="PSUM"))
                ptp = att_sc.enter_context(tc.tile_pool(name="ptp", bufs=4))
                dnp = att_sc.enter_context(tc.tile_pool(name="dnp", bufs=2))
                bcp = att_sc.enter_context(tc.tile_pool(name="bcp", bufs=4))
                # stp last so it can close first (LIFO) for the drain reuse
                stp_sc = ExitStack()
                stp = stp_sc.enter_context(
                    tc.tile_pool(name="stp", bufs=5, space="PSUM"))

                wo_at = [None] * ND

                # per-k-tile query sub-ranges: j=0 has no valid keys for
                # queries >=128 and j=3 none below (skip those score cols;
                # the bias pass overwrites the unwritten PSUM via the bank
                # pending-zero), j=2's violations live only in cols <128.
                # j=0/1 bias stays full-width to cover chunk-0 validity.
                SC_R = [(0, 128), (0, 256), (0, 256), (128, 256)]
                BI_R = [(0, 256), (0, 256), (0, 128), (0, 256)]

                def scores_pair(qh, hp):
                    kt0 = qh * 2
                    qs0 = qh * 256
                    poh = hp // 2
                    pt = ptp.tile([P, 8, 256], f8, name="pt")
                    for w in range(2):
                        off = w * DH
                        for g in range(2):
                            st = stp.tile([P, 2, 256], f32, name="st")
                            for jj in range(2):
                                j = 2 * g + jj
                                (s0, s1), (b0, b1) = SC_R[j], BI_R[j]
                                nc.tensor.matmul(
                                    st[:, jj, s0:s1],
                                    kT[off:off + DH, poh,
                                       (kt0 + j) * P:(kt0 + j + 1) * P],
                                    qT[off:off + DH, poh,
                                       qs0 + s0:qs0 + s1],
                                    start=True, stop=False)
                                # window mask as -1e10 triangle bias
                                nc.tensor.matmul(
                                    st[:, jj, b0:b1], iden[:],
                                    maskT[:, qh, j, b0:b1],
                                    start=False, stop=True)
                            nc.scalar.activation(
                                pt[:, 4 * w + 2 * g:4 * w + 2 * g + 2, :],
                                st[:], AF.Exp, bias=shiftb[:],
                                scale=sc["EXPS"])
                    return pt

                def consume_pair(qh, hp, pt):
                    kt0 = qh * 2
                    qs = slice(qh * 256, (qh + 1) * 256)
                    poh = hp // 2
                    # heads A/B in free slabs, all matmul outs at partition 0
                    # (dual-fp8 matmuls require dst partition base 0); the
                    # [:,2:4,:] half holds 64 replicated denominator rows
                    av = avp.tile([DH, 4, 256], f32, name="av")
                    for w in range(2):
                        for j in range(2):
                            nc.tensor.matmul(
                                av[:, w, :],
                                vaug[:, kt0 + 2 * j:kt0 + 2 * j + 2, hp + w, :],
                                pt[:, 4 * w + 2 * j:4 * w + 2 * j + 2, :],
                                start=(j == 0), stop=(j == 1), perf_mode=DR)
                        for j in range(2):
                            nc.tensor.matmul(
                                av[:, 2 + w, :], onesb[:],
                                pt[:, 4 * w + 2 * j:4 * w + 2 * j + 2, :],
                                start=(j == 0), stop=(j == 1), perf_mode=DR)
                    # one reciprocal covers both heads' denominator blocks;
                    # PSUM x SBUF muls may shift the out partition base, the
                    # final SBUF x SBUF gate mul is base-aligned full-width
                    srw = dnp.tile([DH, 2, 256], bf, name="srw")
                    nc.vector.reciprocal(srw[:], av[:, 2:4, :])
                    tmp = bcp.tile([P, 256], bf, name="tmp")
                    nc.vector.tensor_mul(tmp[0:DH, :], av[:, 0, :],
                                         srw[:, 0, :])
                    nc.vector.tensor_mul(tmp[DH:P, :], av[:, 1, :],
                                         srw[:, 1, :])
                    nc.gpsimd.tensor_mul(at8T[:, poh, qs], tmp[:],
                                         gateT[:, poh, qs])

                po_pair = [None]

                def oproj_chain(pjx, half):
                    qs = slice(half * 256, (half + 1) * 256)
                    if pjx % 2 == 0:
                        po_pair[0] = pop.tile([P, 2, 256], f32, name="po",
                                              tag="po")
                    t = po_pair[0][:, pjx % 2, :]
                    for po in range(0, ND, 2):
                        nc.tensor.matmul(t, wo_at[pjx][:, po:po + 2, :],
                                         at8T[:, po:po + 2, qs],
                                         start=(po == 0), stop=(po == ND - 2),
                                         perf_mode=DR)
                    if pjx % 2 == 1:
                        # s_o*x2 = s_o*x + psum for the pair, in one DVE op
                        nc.vector.tensor_add(xF[:, pjx - 1:pjx + 1, qs],
                                             po_pair[0][:],
                                             xF[:, pjx - 1:pjx + 1, qs])

                pairs = ([(0, hp) for hp in range(0, NH, 2)]
                         + [(1, hp) for hp in range(0, NH, 2)])
                # half-0 out-proj chains distributed over iters 10..15
                opch = {10: [0], 11: [1], 12: [2, 3], 13: [4, 5],
                        14: [6], 15: [7]}
                pend = []
                for i, (qh, hp) in enumerate(pairs):
                    pend.append((qh, hp, scores_pair(qh, hp)))
                    if len(pend) > 2:
                        consume_pair(*pend.pop(0))
                    if i < 4:
                        dma(xF[:, 2 * i:2 * i + 2, :],
                            io["xf"][:, 2 * i:2 * i + 2, :], eng=nc.sync)
                        for h2_ in (2 * i, 2 * i + 1):
                            wo_at[h2_] = wsp4.tile([P, ND, P], f8, name="wt4")
                            dma(wo_at[h2_][:], io["wo_a"][:, h2_],
                                eng=nc.sync)
                    elif i >= 8:
                        for pjx in opch.get(i, ()):
                            oproj_chain(pjx, 0)
                        if i < 12:
                            gu_fetch(i - 8)
                # drain: half-1 out-proj chains overlap the last two pairs'
                # softmax tails -- contraction slabs 0..5 (ready) run first,
                # slabs 6..7 wait only on the final pair's at8
                consume_pair(*pend.pop(0))
                stp_sc.close()  # scores PSUM freed (WARs tracked per bank)
                po1p = att_sc.enter_context(
                    tc.tile_pool(name="po1p", bufs=1, space="PSUM"))
                # two chains share each bank and their starts interleave, so
                # a start=True would wipe the sibling's partials (PSUM
                # auto-zero is bank-granular): memset + accumulate-only
                p1t = [po1p.tile([P, 2, 256], f32, name=f"p1_{j}",
                                 tag=f"p1_{j}") for j in range(4)]
                for j in range(4):
                    nc.vector.memset(p1t[j][:], 0.0)
                qs1 = slice(256, 512)
                for po in range(0, 6, 2):
                    if po >= 2 and pend:
                        consume_pair(*pend.pop(0))
                    for pjx in range(ND):
                        nc.tensor.matmul(p1t[pjx // 2][:, pjx % 2, :],
                                         wo_at[pjx][:, po:po + 2, :],
                                         at8T[:, po:po + 2, qs1],
                                         start=False, stop=False,
                                         perf_mode=DR,
                                         skip_group_check=True)
                while pend:
                    consume_pair(*pend.pop(0))
                msp5 = att_sc.enter_context(
                    tc.tile_pool(name="msp5", bufs=1, space="PSUM"))
                ms5 = msp5.tile([1, CHUNK], f32, name="ms5")
                for pjx in range(ND):
                    nc.tensor.matmul(p1t[pjx // 2][:, pjx % 2, :],
                                     wo_at[pjx][:, 6:8, :],
                                     at8T[:, 6:8, qs1],
                                     start=False, stop=True, perf_mode=DR,
                                     skip_group_check=True)
                    if pjx % 2 == 0:
                        continue
                    nc.vector.tensor_add(xF[:, pjx - 1:pjx + 1, qs1],
                                         p1t[pjx // 2][:],
                                         xF[:, pjx - 1:pjx + 1, qs1])
                    # rms2 square + reduce as each s_o*x2 row-pair lands
                    sq = sqp5.tile([P, 2, CHUNK], f32, name="sq5")
                    nc.scalar.activation(sq[:], xF[:, pjx - 1:pjx + 1, :],
                                         AF.Square)
                    for w in range(2):
                        nc.tensor.matmul(ms5[:], onescol_f[:], sq[:, w, :],
                                         start=(pjx == 1 and w == 0),
                                         stop=(pjx == ND - 1 and w == 1))

                sr5 = r2p.tile([1, CHUNK], f32, name="sr5")
                nc.scalar.activation(sr5[:], ms5[:], AF.Sqrt,
                                     bias=eps2b[0:1, :], scale=1.0 / D)
                nc.vector.reciprocal(rro2[:], sr5[:])
                att_sc.close()  # frees the attention + drain PSUM banks

            tap("dbg_at8", at8T[:])

        # xF = s_o*x2 from here on.
        if "dbg_x2" in io:
            nc.sync.dma_start(out=io["dbg_x2"], in_=xF[:])

        # ---- phase 5: rmsnorm2 broadcast + swiglu ffn (bf16) -------------
        with ExitStack() as fctx:
            h2T = fctx.enter_context(tc.tile_pool(name="h2Tp", bufs=1)).tile(
                [P, ND, CHUNK], bf, name="h2T")
            prod = fctx.enter_context(tc.tile_pool(name="prodp", bufs=1)).tile(
                [P, NF, CHUNK], bf, name="prod")

            with ExitStack() as pctx:
                rbp = pctx.enter_context(
                    tc.tile_pool(name="rbp5", bufs=1, space="PSUM"))
                rowp = pctx.enter_context(tc.tile_pool(name="rowp5", bufs=1))

                rb = rbp.tile([P, CHUNK], f32, name="rb5")
                # consts row 1 = 1.0 (h2 must land unscaled for bf16 ffn)
                nc.tensor.matmul(rb[:], consts[0:1, 1, :], rro2[:],
                                 start=True, stop=True)
                # duplicated so h2 muls cover two a-tiles per op; DVE/Pool
                rbc = rowp.tile([P, 2, CHUNK], bf, name="rbc5")
                nc.vector.tensor_copy(rbc[:, 0, :], rb[:])
                nc.scalar.copy(rbc[:, 1, :], rb[:])
                for a in range(0, ND, 2):
                    eng = nc.vector if (a // 2) % 2 == 0 else nc.gpsimd
                    eng.tensor_mul(h2T[:, a:a + 2, :], xF[:, a:a + 2, :],
                                   rbc[:])

            tap("dbg_h2", h2T[:])

            # bf16 copy of s_o*x2 for the PE-side residual accumulation
            # (emitted mid-g/u: only the final iden matmuls consume it)
            xFb = fctx.enter_context(tc.tile_pool(name="xFbp", bufs=1)).tile(
                [P, ND, CHUNK], bf, name="xFb")

            # gate/up products: fo-pairs share PSUM tiles -> one silu + one
            # prod mult per pair, prod mults alternating DVE/Pool
            with ExitStack() as pctx:
                pg = pctx.enter_context(
                    tc.tile_pool(name="pg", bufs=2, space="PSUM"))
                pu = pctx.enter_context(
                    tc.tile_pool(name="pu", bufs=2, space="PSUM"))
                sgp = pctx.enter_context(tc.tile_pool(name="sgp", bufs=3))

                WOORD = [*range(2, NF), 0, 1]
                for fo in range(0, NF, 2):
                    if 8 <= fo < 16:
                        a = (fo - 8) // 2 * 2
                        eng = nc.vector if (a // 2) % 2 == 0 else nc.gpsimd
                        eng.tensor_copy(xFb[:, a:a + 2, :], xF[:, a:a + 2, :])
                    for w in range(2):
                        if fo + w >= FPRE:
                            gu_fetch(fo + w)
                        if fo + w >= NF - 4:
                            wo_fetch(WOORD[fo + w - (NF - 4)])
                    gps = pg.tile([P, 2, CHUNK], f32, name="gps")
                    for w in range(2):
                        for pi in range(ND):
                            nc.tensor.matmul(gps[:, w, :],
                                             wg_tiles[fo + w][:, pi, :],
                                             h2T[:, pi, :],
                                             start=(pi == 0),
                                             stop=(pi == ND - 1))
                    sg = sgp.tile([P, 2, CHUNK], bf, name="sg")
                    nc.scalar.activation(sg[:], gps[:], AF.Silu)
                    ups = pu.tile([P, 2, CHUNK], f32, name="ups")
                    for w in range(2):
                        for pi in range(ND):
                            nc.tensor.matmul(ups[:, w, :],
                                             wu_tiles[fo + w][:, pi, :],
                                             h2T[:, pi, :],
                                             start=(pi == 0),
                                             stop=(pi == ND - 1))
                    nc.vector.tensor_mul(prod[:, fo:fo + 2, :], sg[:], ups[:])

            # wo: out accumulation over all fo
            with ExitStack() as pctx:
                pz = pctx.enter_context(
                    tc.tile_pool(name="pz", bufs=1, space="PSUM"))
                outp = pctx.enter_context(tc.tile_pool(name="outp", bufs=1))

                z2 = [pz.tile([P, CHUNK], f32, name=f"z2_{j}", tag=f"z2_{j}")
                      for j in range(ND)]
                ot = outp.tile([P, ND, CHUNK], bf, name="ot")
                # fo order rotated so the final rounds use early-produced
                # prod tiles (the last prod mults are still in flight)
                outT_p = io["outT"].rearrange("(a p) t -> p a t", p=P)
                for r, fo in enumerate(WOORD[:NF - 4]):
                    if r + 4 < NF:
                        wo_fetch(WOORD[r + 4])
                    wof = wo_tiles[fo]
                    for j in range(ND):
                        nc.tensor.matmul(z2[j][:], wof[:, j * P:(j + 1) * P],
                                         prod[:, fo, :],
                                         start=(r == 0), stop=False)
                # last 4 rounds j-major: early columns close + copy + DMA
                # while later columns still accumulate (shrinks the tail)
                for j in range(ND):
                    for fo in WOORD[NF - 4:]:
                        nc.tensor.matmul(z2[j][:],
                                         wo_tiles[fo][:, j * P:(j + 1) * P],
                                         prod[:, fo, :],
                                         start=False, stop=False)
                    # residual via (1/s_o)-diagonal matmul closes the group
                    nc.tensor.matmul(z2[j][:], iden[:], xFb[:, j, :],
                                     start=False, stop=True)
                    if j % 2 == 0:
                        nc.vector.tensor_copy(ot[:, j, :], z2[j][:])
                    else:
                        nc.scalar.copy(ot[:, j, :], z2[j][:])
                    for hh in range(2):
                        # early j may use the slower SWDGE (hidden under the
                        # remaining matmuls); the last columns stay on SP
                        eng = nc.gpsimd if (hh == 1 and j < 6) else nc.sync
                        dma(outT_p[:, j:j + 1, hh * 256:(hh + 1) * 256],
                            ot[:, j, hh * 256:(hh + 1) * 256], eng=eng)


# --------------------------------------------------------------------------
# host-side sharding / unsharding
# --------------------------------------------------------------------------

def _bf16(x):
    import ml_dtypes
    return np.ascontiguousarray(x.astype(ml_dtypes.bfloat16))


def _f8(x):
    import ml_dtypes
    return np.ascontiguousarray(x.astype(ml_dtypes.float8_e4m3))


def _pow2floor(v):
    return 2.0 ** np.floor(np.log2(v))


def _tile_kmajor(w):
    """[D_in, D_out] -> [P, D_in//P (po-tiles of 128 out-cols), ...] layout
    [p, po, a, o] where w[a*P+p, po*P+o]."""
    din, dout = w.shape
    a, po = din // P, dout // P
    return np.ascontiguousarray(
        w.reshape(a, P, po, P).transpose(1, 2, 0, 3))


def _build_trimask(chunk_start, neg_bias):
    """Violation-bias consts in [s_partition, qhalf, ktile, r] layout.
    The on-device matmul computes st[p, r] += (1/s_o) * trimask[p, qh, j, r];
    trimask carries -s_o*C at (band violation | invalid key), 0 elsewhere."""
    m = np.zeros((2, 4, P, 256), np.float32)
    for qh in range(2):
        c = (np.arange(4 * P)[:, None])            # window key coord [0, 512)
        rr = np.arange(256)[None, :]
        viol = ~((c >= rr + 1) & (c <= rr + WIN))
        invalid = (chunk_start - 256 + qh * 256 + c) < 0
        m[qh] = (viol | invalid).astype(np.float32).reshape(4, P, 256)
    m = m * neg_bias
    return np.ascontiguousarray(m.transpose(2, 0, 1, 3))  # [P, 2, 4, 256]


def compute_scales(qkv_w, gate_w, out_w, ln1_w):
    """Pow2 fold constants from the weights (host side, exact)."""
    tot = NH * DH
    wq_e = (qkv_w[0 * tot:1 * tot] * ln1_w[None, :]).T  # [D, tot]
    wk_e = (qkv_w[1 * tot:2 * tot] * ln1_w[None, :]).T
    wv_e = (qkv_w[2 * tot:3 * tot] * ln1_w[None, :]).T
    wgate_e = (gate_w * ln1_w[None, :]).T
    wout_e = out_w.T                                    # [tot, D]

    m_q = np.abs(wq_e).max(0) + 1e-30                   # per out-channel d
    m_k = np.abs(wk_e).max(0) + 1e-30
    c_glob = _pow2floor((TGT * TGT / (m_q * m_k)).min())
    s_d = _pow2floor(np.sqrt(c_glob * m_k / m_q))
    s_v = _pow2floor(TGT / np.abs(wv_e).max())
    s_g = _pow2floor(TGT / np.abs(wgate_e).max())
    s_o = _pow2floor(TGT / np.abs(wout_e).max())
    return {
        "c_glob": float(c_glob), "s_d": s_d, "s_v": float(s_v),
        "s_g": float(s_g), "s_o": float(s_o),
        "EXPS": float(SCALE / (SA * SA * c_glob)),
        "SIG": float(1.0 / (SA * s_g)),
        "KV": float(SV8 / (SA * s_v)),
        "EPS2": float(EPS * s_o * s_o),
    }


def make_in_maps(x, ln1_w, qkv_w, gate_w, out_w, ln2_w, wg, wu, wo, sc):
    tot = NH * DH
    # fold rmsnorm weights into the consuming projection weights
    wq_e = (qkv_w[0 * tot:1 * tot] * ln1_w[None, :]).T  # [D(in), D(out)]
    wk_e = (qkv_w[1 * tot:2 * tot] * ln1_w[None, :]).T
    wv_e = (qkv_w[2 * tot:3 * tot] * ln1_w[None, :]).T
    wgate_e = (gate_w * ln1_w[None, :]).T
    wout_e = out_w.T                                    # [tot, D]
    wg_e = (wg * ln2_w[None, :]).T                      # [D, DFF]
    wu_e = (wu * ln2_w[None, :]).T
    wo_e = wo.T                                         # [DFF, D]

    # fp8 quantization with folded scales
    wq8 = wq_e * sc["s_d"][None, :]
    wk8 = wk_e * (sc["c_glob"] / sc["s_d"])[None, :]
    wv8 = wv_e * sc["s_v"]
    wgt8 = wgate_e * sc["s_g"]
    wout8 = wout_e * sc["s_o"]

    # pre-tiled device layouts
    wv_l = _tile_kmajor(wv8).reshape(P, 4, 2, ND, P).transpose(
        0, 1, 3, 2, 4).reshape(P, 4, ND, 256)  # [p, ng, a, 256]
    wg_l = _tile_kmajor(wg_e)                            # [p, fo, a, o]
    wu_l = _tile_kmajor(wu_e)
    wo_l = np.ascontiguousarray(
        wo_e.reshape(NF, P, D).transpose(1, 0, 2))       # [p, fo, d]

    # both rows 1.0: rms1's dequant is deferred (raw fp8 x feeds the
    # projections), rms2's h2 lands unscaled for the bf16 ffn
    consts = np.ones((1, 2, P), np.float32)

    shared = {
        "wq": _f8(_tile_kmajor(wq8)),
        "wk": _f8(_tile_kmajor(wk8)),
        "wv": _f8(wv_l),
        "wgt": _f8(_tile_kmajor(wgt8)),
        "wo_a": _f8(_tile_kmajor(wout8)),
        "wg": _bf16(wg_l),
        "wu": _bf16(wu_l),
        "wo": _bf16(wo_l),
        "consts": _bf16(consts),
        "iden": _bf16(np.eye(P, dtype=np.float32) / sc["s_o"]),
    }

    in_maps = []
    for c in range(NCORES):
        b, ck = divmod(c, T // CHUNK)
        cs = ck * CHUNK
        xw = np.zeros((LT, D), np.float32)
        lo = cs - HALO
        xw[max(0, -lo):] = x[b, max(lo, 0):cs + CHUNK]
        m = dict(shared)
        xt = np.ascontiguousarray(xw.T)                  # [D, LT]
        m["xb"] = _f8((xt * SA).reshape(ND, P, LT).transpose(1, 0, 2))
        m["xf"] = np.ascontiguousarray(
            (xt[:, HALO:] * sc["s_o"]).reshape(ND, P, CHUNK).transpose(1, 0, 2))
        m["trimask"] = _bf16(_build_trimask(cs, -30.0 / sc["EXPS"] * sc["s_o"]))
        in_maps.append(m)
    return in_maps


def gather_output(results):
    out = np.empty((B, T, D), np.float32)
    for c in range(NCORES):
        b, ck = divmod(c, T // CHUNK)
        out[b, ck * CHUNK:(ck + 1) * CHUNK] = results[c]["outT"].T
    return out


def kernel(**inputs):
    from concourse.bass_utils import run_bass_kernel_spmd

    sc = compute_scales(inputs["qkv_w"], inputs["gate_w"], inputs["out_w"],
                        inputs["ln1_w"])
    if "nc" not in _CACHE:
        _CACHE["nc"] = build_program(sc)
        _CACHE["sc"] = sc
    nc = _CACHE["nc"]

    in_maps = make_in_maps(**inputs, sc=sc)
    res = run_bass_kernel_spmd(nc, in_maps, core_ids=list(range(NCORES)))
    return gather_output(res.results)


if __name__ == "__main__":
    rng = np.random.default_rng(0)
    ins = {
        "x": rng.standard_normal((B, T, D), dtype=np.float32),
        "ln1_w": np.ones(D, np.float32),
        "qkv_w": rng.standard_normal((3 * NH * DH, D), dtype=np.float32) * 0.02,
        "gate_w": rng.standard_normal((NH * DH, D), dtype=np.float32) * 0.04,
        "out_w": rng.standard_normal((D, NH * DH), dtype=np.float32) * 0.04,
        "ln2_w": np.ones(D, np.float32),
        "wg": rng.standard_normal((DFF, D), dtype=np.float32) * 0.02,
        "wu": rng.standard_normal((DFF, D), dtype=np.float32) * 0.02,
        "wo": rng.standard_normal((D, DFF), dtype=np.float32) * 0.02,
    }
    out = kernel(**ins)
    print("out", out.shape, out.dtype, float(np.abs(out).mean()))


# revision 90
# speedup vs baseline: 1.8749x; 1.8749x over previous
"""Trainium2 Bass kernel for a sparse-attention (sliding-window) transformer block.

Reference computation (per batch b, token t):
    x = x + attn(rmsnorm(x, ln1_w));  attn = (windowed_softmax(qk)·v * sigmoid(gate)) @ out_w.T
    out = x + swiglu_ffn(rmsnorm(x, ln2_w))

Sharding: token-parallel across 8 cores (B=2 x 4 chunks of 512 tokens).  Each
core receives its 512 tokens plus the previous 256 tokens (sliding-window halo,
zeros for the first chunk) and recomputes K/V on the halo -> no collectives.

v3 versus v2 (~338us TimelineSim): fp8-e4m3 DoubleRow matmuls for the
attention-side GEMMs (q/k/v/gate projections, AV, out-projection) at 2x PE
throughput; scores (contraction 64) and the whole FFN stay bf16 -- fp8 noise
on the FFN branch alone costs ~2.4e-2 relative error (75% of output norm),
while the attention side costs ~8e-3 (CPU-sim measured).  All quantization
scales are powers of two computed from the weights at runtime and folded
into existing instructions:
  - balanced per-channel q/k scales s_d, (c/s_d) cancel in q.k up to the
    global c, which folds into the exp() scale argument,
  - v dequant folds into the PSUM->SBUF copy scale; the fp8 ones-column
    (value SV8) makes the softmax divide remove v's fp8 scale for free,
  - gate dequant folds into the sigmoid scale,
  - the out-projection scale s_o stays ON the residual stream: xf is
    pre-scaled by s_o on the host, rms2's eps becomes eps*s_o^2 (rmsnorm is
    scale-invariant), and the final identity-residual matmul uses a 1/s_o
    diagonal so the output lands unscaled.
"""

import os
import sys

import numpy as np

if "/opt/trn_rl_repo" not in sys.path:
    sys.path.insert(0, "/opt/trn_rl_repo")

# ---- problem constants (hardcoded; kernel.py must be self-contained) ----
D = 1024          # d_model
NH = 16           # heads
DH = 64           # head dim
DFF = 4096        # ffn hidden
WIN = 256         # sliding window
B, T = 2, 2048
EPS = 1e-6

NCORES = 8
CHUNK = 512       # own tokens per core
HALO = 256        # preceding-token halo
LT = CHUNK + HALO  # 768 local tokens (halo first)
P = 128
ND = D // P       # 8
NF = DFF // P     # 32

SCALE = DH ** -0.5
SA = 16.0         # fp8 h1 activation scale
SV8 = 16.0        # fp8 v scale
TGT = 112.0       # fp8 weight absmax target (e4m3 max 240)
EXP_SHIFT = -4.0  # constant shift inside exp; softmax-invariant

_CACHE = {}


# --------------------------------------------------------------------------
# program builder
# --------------------------------------------------------------------------

def build_program(sc):
    """sc: dict of scalar fold constants (EXPS, SIG, KV, EPS2, from weights)."""
    import concourse.bacc as bacc
    import concourse.tile as tile
    from concourse import mybir

    f32 = mybir.dt.float32
    bf = mybir.dt.bfloat16
    f8 = mybir.dt.float8e4

    nc = bacc.Bacc("TRN2", target_bir_lowering=False, debug=False,
                   num_devices=NCORES)

    io = {}
    # activations (xb = fp8 of SA*x, raw; rmsnorm folds into PSUM copies)
    io["xb"] = nc.dram_tensor("xb", [P, ND, LT], f8, kind="ExternalInput").ap()
    io["xf"] = nc.dram_tensor("xf", [P, ND, CHUNK], f32,
                              kind="ExternalInput").ap()
    # attention weights (fp8), pre-tiled [partition, tiles..., free]
    io["wq"] = nc.dram_tensor("wq", [P, ND, ND, P], f8, kind="ExternalInput").ap()
    io["wk"] = nc.dram_tensor("wk", [P, ND, ND, P], f8, kind="ExternalInput").ap()
    io["wv"] = nc.dram_tensor("wv", [P, 4, ND, 256], f8, kind="ExternalInput").ap()
    io["wgt"] = nc.dram_tensor("wgt", [P, ND, ND, P], f8, kind="ExternalInput").ap()
    io["wo_a"] = nc.dram_tensor("wo_a", [P, ND, ND, P], f8, kind="ExternalInput").ap()
    # ffn weights (bf16)
    io["wg"] = nc.dram_tensor("wg", [P, NF, ND, P], bf, kind="ExternalInput").ap()
    io["wu"] = nc.dram_tensor("wu", [P, NF, ND, P], bf, kind="ExternalInput").ap()
    io["wo"] = nc.dram_tensor("wo", [P, NF, D], bf, kind="ExternalInput").ap()
    # triangle bias consts: the sliding-window mask folds into the score
    # PSUM accumulation as one identity-stationary matmul per k-tile,
    # pushing masked logits to -1e10 so exp flushes them to fp8 zero
    io["trimask"] = nc.dram_tensor("trimask", [P, 2, 4, 256], bf,
                                   kind="ExternalInput").ap()
    io["iden"] = nc.dram_tensor("iden", [P, P], bf, kind="ExternalInput").ap()
    io["consts"] = nc.dram_tensor("consts", [1, 2, P], bf,
                                  kind="ExternalInput").ap()
    io["outT"] = nc.dram_tensor("outT", [D, CHUNK], bf,
                                kind="ExternalOutput").ap()

    if os.environ.get("BASS_TAPS") == "1":
        for nm, shape, dt in [
                ("dbg_q", [P, ND, CHUNK], bf),
                ("dbg_k", [P, ND, LT], bf),
                ("dbg_v", [P, (LT // P) * NH * DH], f8),
                ("dbg_gate", [P, ND, CHUNK], bf),
                ("dbg_at8", [P, ND, CHUNK], f8),
                ("dbg_x2", [P, ND, CHUNK], f32),
                ("dbg_h2", [P, ND, CHUNK], bf)]:
            io[nm] = nc.dram_tensor(nm, shape, dt, kind="ExternalOutput").ap()

    with tile.TileContext(nc) as tc:
        _emit(tc, io, sc)

    nc.compile()
    return nc


def _emit(tc, io, sc):
    from contextlib import ExitStack

    from concourse import mybir

    nc = tc.nc
    f32 = mybir.dt.float32
    bf = mybir.dt.bfloat16
    f8 = mybir.dt.float8e4
    AF = mybir.ActivationFunctionType
    DR = mybir.MatmulPerfMode.DoubleRow

    # round-robin DMA issue across the three queues; eng= overrides
    _q = [0]

    def dma(out, in_, eng=None):
        if eng is None:
            eng = (nc.sync, nc.scalar, nc.gpsimd)[_q[0] % 3]
            _q[0] += 1
        eng.dma_start(out=out, in_=in_)

    def tap(name, src_ap):
        if name in io:
            nc.sync.dma_start(out=io[name], in_=src_ap)

    FPRE = 4  # ffn fo-tiles prefetched during attention

    with ExitStack() as ctx:
        ctx.enter_context(nc.allow_low_precision(
            reason="fp8/bf16 matmul inputs; all accumulation stays fp32 PSUM"))
        glob = ctx.enter_context(tc.tile_pool(name="glob", bufs=1))

        # ---- phase 0: global constants + input prefetch ------------------
        consts = glob.tile([1, 2, P], bf, name="consts")
        dma(consts[:], io["consts"], eng=nc.sync)
        xTp = ctx.enter_context(tc.tile_pool(name="xTp", bufs=1))
        xT = xTp.tile([P, ND, LT], f8, name="xT")
        xb_engs = (nc.sync, nc.scalar, nc.gpsimd, nc.sync)
        for i, a in enumerate(range(0, ND, 2)):
            dma(xT[:, a:a + 2, :], io["xb"][:, a:a + 2, :], eng=xb_engs[i])
        maskT = glob.tile([P, 2, 4, 256], bf, name="maskT")

        onescol_f = glob.tile([P, 1], f32)
        nc.vector.memset(onescol_f, 1.0)
        onescol = glob.tile([P, 1], bf)
        nc.vector.tensor_copy(onescol[:], onescol_f[:])
        epsb = glob.tile([P, 1], f32)
        nc.vector.memset(epsb, EPS)
        eps2b = glob.tile([P, 1], f32)
        nc.vector.memset(eps2b, sc["EPS2"])
        shiftb = glob.tile([P, 1], f32)
        nc.vector.memset(shiftb, EXP_SHIFT)

        # prewarm the Sqrt table (rms1 critical path); the other tables are
        # warmed right after the rms sqrt so they never delay it
        warm = glob.tile([1, 1], f32, name="warm")
        nc.scalar.activation(warm[:], epsb[0:1, :], AF.Sqrt)

        # residual fp32 s_o*x (host-prescaled; loaded at out-proj time)
        xF = glob.tile([P, ND, CHUNK], f32, name="xF")

        # identity/s_o stationary for PE-side residual accumulation
        iden = glob.tile([P, P], bf, name="iden")
        dma(iden[:], io["iden"], eng=nc.sync)
        # trimask is needed only once attention starts; keep it off the Act
        # queue (its dma_start issue would delay the rms sqrt)
        dma(maskT[:], io["trimask"], eng=nc.sync)

        # ffn weights: persistent pools; wg/wu prefetch FPRE tiles early,
        # wo streams within the wo pass (bufs ahead) to avoid recycle stalls
        wgp = ctx.enter_context(tc.tile_pool(name="wgp", bufs=FPRE + 1))
        wup = ctx.enter_context(tc.tile_pool(name="wup", bufs=FPRE + 1))
        wop = ctx.enter_context(tc.tile_pool(name="wop", bufs=4))
        wg_tiles, wu_tiles, wo_tiles = {}, {}, {}

        # ffn weight streaming stays off the Act queue (exp/silu live there)
        def gu_fetch(fo):
            wg_tiles[fo] = wgp.tile([P, ND, P], bf, name="wgf")
            dma(wg_tiles[fo][:], io["wg"][:, fo], eng=nc.sync)
            wu_tiles[fo] = wup.tile([P, ND, P], bf, name="wuf")
            dma(wu_tiles[fo][:], io["wu"][:, fo], eng=nc.gpsimd)

        def wo_fetch(fo):
            wo_tiles[fo] = wop.tile([P, D], bf, name="wof")
            dma(wo_tiles[fo][:], io["wo"][:, fo],
                eng=(nc.sync, nc.gpsimd)[fo % 2])

        # ---- phase 1: rmsnorm1 over all LT tokens ------------------------
        with ExitStack() as actx:
            qT = actx.enter_context(tc.tile_pool(name="qTp", bufs=1)).tile(
                [P, ND, CHUNK], bf, name="qT")
            kT = actx.enter_context(tc.tile_pool(name="kTp", bufs=1)).tile(
                [P, ND, LT], bf, name="kT")
            vaug = actx.enter_context(tc.tile_pool(name="vaugp", bufs=1)).tile(
                [P, LT // P, NH, DH], f8, name="vaug")
            gateT = actx.enter_context(tc.tile_pool(name="gateTp", bufs=1)).tile(
                [P, ND, CHUNK], bf, name="gateT")
            at8T = actx.enter_context(tc.tile_pool(name="at8Tp", bufs=1)).tile(
                [P, ND, CHUNK], f8, name="at8T")

            # projection weights: resident for phases 1-2 only
            wvt = actx.enter_context(tc.tile_pool(name="wvtp", bufs=1)).tile(
                [P, 4, ND, 256], f8, name="wvt")
            deqp = actx.enter_context(tc.tile_pool(name="deqp", bufs=1))
            rbc = deqp.tile([P, 2, LT], bf, name="rbc")
            rvk = deqp.tile([P, 6], f32, name="rvk")
            sgp2 = actx.enter_context(tc.tile_pool(name="sgp2", bufs=4))
            projw_sc = ExitStack()
            projw = projw_sc.enter_context(tc.tile_pool(name="projw", bufs=1))
            wqt = projw.tile([P, ND, ND, P], f8, name="wqt")
            wkt = projw.tile([P, ND, ND, P], f8, name="wkt")
            wgtt = projw.tile([P, ND, ND, P], f8, name="wgtt")
            # first-use order: q, k, v, gate
            for dst, src in ((wqt, "wq"), (wkt, "wk")):
                dma(dst[:, 0:4], io[src][:, 0:4], eng=nc.sync)
                dma(dst[:, 4:8], io[src][:, 4:8], eng=nc.gpsimd)
            dma(wvt[:, 0:2], io["wv"][:, 0:2], eng=nc.sync)
            dma(wvt[:, 2:4], io["wv"][:, 2:4], eng=nc.gpsimd)
            dma(wgtt[:, 0:4], io["wgt"][:, 0:4], eng=nc.sync)
            dma(wgtt[:, 4:8], io["wgt"][:, 4:8], eng=nc.gpsimd)

            with ExitStack() as pctx:
                sqp = pctx.enter_context(tc.tile_pool(name="sqp", bufs=2))
                msp = pctx.enter_context(
                    tc.tile_pool(name="msp", bufs=2, space="PSUM"))
                rbp = pctx.enter_context(
                    tc.tile_pool(name="rbp", bufs=2, space="PSUM"))
                rowp = pctx.enter_context(tc.tile_pool(name="rowp", bufs=2))

                msg = [msp.tile([1, 384], f32, name=f"ms{g}", tag=f"ms{g}")
                       for g in range(2)]
                for a in range(ND):
                    sq = sqp.tile([P, LT], bf, name="sq")
                    # x is fp8(SA*x); squares split across DVE/Act/Pool
                    if a % 3 == 2:
                        nc.scalar.activation(sq[:], xT[:, a, :], AF.Square)
                    else:
                        eng = nc.vector if a % 3 == 0 else nc.gpsimd
                        eng.tensor_mul(sq[:], xT[:, a, :], xT[:, a, :])
                    for g in range(2):
                        sl = slice(g * 384, (g + 1) * 384)
                        nc.tensor.matmul(msg[g][:], onescol[:], sq[:, sl],
                                         start=(a == 0), stop=(a == ND - 1))
                sr = rowp.tile([1, LT], f32, name="sr")
                for g in range(2):
                    nc.scalar.activation(sr[:, g * 384:(g + 1) * 384],
                                         msg[g][:], AF.Sqrt,
                                         bias=epsb[0:1, :],
                                         scale=1.0 / (D * SA * SA))
                # warm the remaining Act tables now (Act idles during the
                # projection matmuls; swaps would otherwise hit attention)
                for wf in (AF.Exp, AF.Sigmoid, AF.Square, AF.Silu):
                    nc.scalar.activation(warm[:], epsb[0:1, :], wf)
                rro = rowp.tile([1, LT], bf, name="rro")
                nc.vector.reciprocal(rro[:], sr[:])
                # rbc duplicated x2 (pair-merged dequant muls); rmsnorm is
                # deferred through the projections: rbc = 1/rms per token
                for g in range(2):
                    sl = slice(g * 384, (g + 1) * 384)
                    rb = rbp.tile([P, 384], f32, name="rb")
                    nc.tensor.matmul(rb[:], consts[0:1, 0, :], rro[:, sl],
                                     start=True, stop=True)
                    nc.vector.tensor_copy(rbc[:, 0, sl], rb[:])
                    nc.scalar.copy(rbc[:, 1, sl], rb[:])
                # per-token-tile 1/rms columns for the v-projection copies
                # (gpsimd DMA can cast bf16->f32; KV folds into the copies'
                # second tensor_scalar operand, so no extra DVE op here)
                for tt in range(6):
                    nc.gpsimd.dma_start(out=rvk[:, tt:tt + 1],
                                        in_=rro[0:1, tt * P:(tt + 1) * P])

            # fp8 all-SV8 stationary for the denominator-block matmuls (the
            # softmax divide dequantizes v's fp8 scale); 64 replicated rows
            # so the whole reciprocal is one DVE op and no broadcast needed
            onesbf = glob.tile([P, 2, DH], f32)
            nc.vector.memset(onesbf, SV8)
            onesb = glob.tile([P, 2, DH], f8)
            nc.vector.tensor_copy(onesb[:], onesbf[:])


            gp_tiles = []
            # ---- phase 2: q/k/v/gate projections (fp8 DoubleRow) ---------
            # projections consume raw fp8 x; the deferred 1/rms rides the
            # PSUM->SBUF dequant ops (DVE/Pool mults, Act per-partition scale)
            with ExitStack() as pctx:
                pj = pctx.enter_context(
                    tc.tile_pool(name="pj", bufs=4, space="PSUM"))

                # q^T [D, CHUNK] (own tokens only); holds SA*s_d*q_d
                for po in range(0, ND, 2):
                    ps = pj.tile([P, 2, CHUNK], f32, name="ps", tag="pj")
                    for w in range(2):
                        for pi in range(0, ND, 2):
                            nc.tensor.matmul(ps[:, w, :],
                                             wqt[:, po + w, pi:pi + 2, :],
                                             xT[:, pi:pi + 2, HALO:LT],
                                             start=(pi == 0),
                                             stop=(pi == ND - 2),
                                             perf_mode=DR)
                    nc.vector.tensor_mul(qT[:, po:po + 2, :], ps[:],
                                         rbc[:, :, HALO:LT])

                # k^T [D, LT]; holds SA*(c/s_d)*k_d.  PSUM slabs are padded
                # to 2KB so each accumulation group stays within one bank
                for po in range(ND):
                    ps = pj.tile([P, 2, CHUNK], f32, name="psk", tag="pj")
                    for g in range(2):
                        sl = slice(g * 384, (g + 1) * 384)
                        for pi in range(0, ND, 2):
                            nc.tensor.matmul(ps[:, g, 0:384],
                                             wkt[:, po, pi:pi + 2, :],
                                             xT[:, pi:pi + 2, sl],
                                             start=(pi == 0),
                                             stop=(pi == ND - 2),
                                             perf_mode=DR)
                    nc.vector.tensor_mul(
                        kT[:, po, :], ps[:, :, 0:384],
                        rbc[:, 0, :].rearrange("p (g t) -> p g t", g=2))

                # v [LT, D], token-major (tokens on PSUM partitions): ng-pair
                # PSUM tiles (heads contiguous in vaug) -> 12 Act copies with
                # the per-partition KV/rms dequant scale
                for tt in range(6):
                    for ng in range(0, 4, 2):
                        ps = pj.tile([P, 2, 256], f32, name="psv", tag="pj")
                        for w in range(2):
                            for pi in range(0, ND, 2):
                                nc.tensor.matmul(
                                    ps[:, w, :],
                                    xT[:, pi:pi + 2, tt * P:(tt + 1) * P],
                                    wvt[:, ng + w, pi:pi + 2, :],
                                    start=(pi == 0), stop=(pi == ND - 2),
                                    perf_mode=DR)
                        # DVE (per-partition scalar) keeps the Act queue
                        # clear so the first exps start right after phase 2
                        nc.vector.tensor_scalar(
                            vaug[:, tt, 4 * ng:4 * (ng + 2), :],
                            ps[:].rearrange("p two (h d) -> p (two h) d",
                                            d=DH),
                            rvk[:, tt:tt + 1], sc["KV"],
                            op0=mybir.AluOpType.mult,
                            op1=mybir.AluOpType.mult)

                # gate^T = sigmoid(r/(SA*s_g) * psum)^T [D, CHUNK]
                for po in range(0, ND, 2):
                    ps = pj.tile([P, 2, CHUNK], f32, name="ps", tag="pj")
                    for w in range(2):
                        for pi in range(0, ND, 2):
                            nc.tensor.matmul(ps[:, w, :],
                                             wgtt[:, po + w, pi:pi + 2, :],
                                             xT[:, pi:pi + 2, HALO:LT],
                                             start=(pi == 0),
                                             stop=(pi == ND - 2),
                                             perf_mode=DR)
                    gp = sgp2.tile([P, 2, CHUNK], bf, name="gp")
                    nc.vector.scalar_tensor_tensor(
                        gp[:], ps[:], sc["SIG"], rbc[:, :, HALO:LT],
                        op0=mybir.AluOpType.mult, op1=mybir.AluOpType.mult)
                    # sigmoid deferred into the attention loop so the Act
                    # queue serves the first exps before the gates
                    gp_tiles.append((po, gp))

            tap("dbg_q", qT[:])
            tap("dbg_k", kT[:])
            tap("dbg_v", vaug[:].rearrange("p a h d -> p (a h d)"))
            tap("dbg_gate", gateT[:])

            projw_sc.close()  # free the projection weights' SBUF

            # ---- phase 3+4: attention, depth-2 software pipeline ----------
            # per pair: PE scores (8 matmuls) -> Act exp (2 ops) -> mask mult
            # split DVE/Pool -> PE AV + denominators (DoubleRow) -> recip ->
            # broadcast -> one fused divide + one gate mult per pair.
            # consume(p-2) runs while Act/DVE chew on pairs p-1/p.
            rro2 = glob.tile([1, CHUNK], bf, name="rro2")
            with ExitStack() as pctx:
                pop = pctx.enter_context(
                    tc.tile_pool(name="pop", bufs=1, space="PSUM"))
                wsp4 = pctx.enter_context(tc.tile_pool(name="wsp4", bufs=8))
                sqp5 = pctx.enter_context(tc.tile_pool(name="sqp5", bufs=2))
                r2p = pctx.enter_context(tc.tile_pool(name="r2p", bufs=1))
                att_sc = ExitStack()
                avp = att_sc.enter_context(
                    tc.tile_pool(name="avp", bufs=1, space="PSUM"))
                ptp = att_sc.enter_context(tc.tile_pool(name="ptp", bufs=4))
                dnp = att_sc.enter_context(tc.tile_pool(name="dnp", bufs=2))
                bcp = att_sc.enter_context(tc.tile_pool(name="bcp", bufs=4))
                # stp last so it can close first (LIFO) for the drain reuse
                stp_sc = ExitStack()
                stp = stp_sc.enter_context(
                    tc.tile_pool(name="stp", bufs=5, space="PSUM"))

                wo_at = [None] * ND

                # per-k-tile query sub-ranges: j=0 has no valid keys for
                # queries >=128 and j=3 none below (skip those score cols;
                # the bias pass overwrites the unwritten PSUM via the bank
                # pending-zero), j=2's violations live only in cols <128.
                # j=0/1 bias stays full-width to cover chunk-0 validity.
                SC_R = [(0, 128), (0, 256), (0, 256), (128, 256)]
                BI_R = [(0, 256), (0, 256), (0, 128), (0, 256)]

                def scores_pair(qh, hp):
                    kt0 = qh * 2
                    qs0 = qh * 256
                    poh = hp // 2
                    pt = ptp.tile([P, 8, 256], f8, name="pt")
                    for w in range(2):
                        off = w * DH
                        for g in range(2):
                            st = stp.tile([P, 2, 256], f32, name="st")
                            for jj in range(2):
                                j = 2 * g + jj
                                (s0, s1), (b0, b1) = SC_R[j], BI_R[j]
                                nc.tensor.matmul(
                                    st[:, jj, s0:s1],
                                    kT[off:off + DH, poh,
                                       (kt0 + j) * P:(kt0 + j + 1) * P],
                                    qT[off:off + DH, poh,
                                       qs0 + s0:qs0 + s1],
                                    start=True, stop=False)
                                # window mask as -1e10 triangle bias
                                nc.tensor.matmul(
                                    st[:, jj, b0:b1], iden[:],
                                    maskT[:, qh, j, b0:b1],
                                    start=False, stop=True)
                            nc.scalar.activation(
                                pt[:, 4 * w + 2 * g:4 * w + 2 * g + 2, :],
                                st[:], AF.Exp, bias=shiftb[:],
                                scale=sc["EXPS"])
                    return pt

                def consume_pair(qh, hp, pt):
                    kt0 = qh * 2
                    qs = slice(qh * 256, (qh + 1) * 256)
                    poh = hp // 2
                    # heads A/B in free slabs, all matmul outs at partition 0
                    # (dual-fp8 matmuls require dst partition base 0); the
                    # [:,2:4,:] half holds 64 replicated denominator rows
                    av = avp.tile([DH, 4, 256], f32, name="av")
                    for w in range(2):
                        for j in range(2):
                            nc.tensor.matmul(
                                av[:, w, :],
                                vaug[:, kt0 + 2 * j:kt0 + 2 * j + 2, hp + w, :],
                                pt[:, 4 * w + 2 * j:4 * w + 2 * j + 2, :],
                                start=(j == 0), stop=(j == 1), perf_mode=DR)
                        for j in range(2):
                            nc.tensor.matmul(
                                av[:, 2 + w, :], onesb[:],
                                pt[:, 4 * w + 2 * j:4 * w + 2 * j + 2, :],
                                start=(j == 0), stop=(j == 1), perf_mode=DR)
                    # one reciprocal covers both heads' denominator blocks;
                    # PSUM x SBUF muls may shift the out partition base, the
                    # final SBUF x SBUF gate mul is base-aligned full-width
                    srw = dnp.tile([DH, 2, 256], bf, name="srw")
                    nc.vector.reciprocal(srw[:], av[:, 2:4, :])
                    tmp = bcp.tile([P, 256], bf, name="tmp")
                    nc.vector.tensor_mul(tmp[0:DH, :], av[:, 0, :],
                                         srw[:, 0, :])
                    nc.vector.tensor_mul(tmp[DH:P, :], av[:, 1, :],
                                         srw[:, 1, :])
                    nc.gpsimd.tensor_mul(at8T[:, poh, qs], tmp[:],
                                         gateT[:, poh, qs])

                po_pair = [None]

                def oproj_chain(pjx, half):
                    qs = slice(half * 256, (half + 1) * 256)
                    if pjx % 2 == 0:
                        po_pair[0] = pop.tile([P, 2, 256], f32, name="po",
                                              tag="po")
                    t = po_pair[0][:, pjx % 2, :]
                    for po in range(0, ND, 2):
                        nc.tensor.matmul(t, wo_at[pjx][:, po:po + 2, :],
                                         at8T[:, po:po + 2, qs],
                                         start=(po == 0), stop=(po == ND - 2),
                                         perf_mode=DR)
                    if pjx % 2 == 1:
                        # s_o*x2 = s_o*x + psum for the pair, in one DVE op
                        nc.vector.tensor_add(xF[:, pjx - 1:pjx + 1, qs],
                                             po_pair[0][:],
                                             xF[:, pjx - 1:pjx + 1, qs])

                pairs = ([(0, hp) for hp in range(0, NH, 2)]
                         + [(1, hp) for hp in range(0, NH, 2)])
                # half-0 out-proj chains distributed over iters 10..15
                opch = {10: [0], 11: [1], 12: [2, 3], 13: [4, 5],
                        14: [6], 15: [7]}
                pend = []
                for i, (qh, hp) in enumerate(pairs):
                    pend.append((qh, hp, scores_pair(qh, hp)))
                    if i < len(gp_tiles):
                        po_g, gp_g = gp_tiles[i]
                        nc.scalar.activation(gateT[:, po_g:po_g + 2, :],
                                             gp_g[:], AF.Sigmoid)
                    if len(pend) > 2:
                        consume_pair(*pend.pop(0))
                    if i < 4:
                        dma(xF[:, 2 * i:2 * i + 2, :],
                            io["xf"][:, 2 * i:2 * i + 2, :], eng=nc.sync)
                        for h2_ in (2 * i, 2 * i + 1):
                            wo_at[h2_] = wsp4.tile([P, ND, P], f8, name="wt4")
                            dma(wo_at[h2_][:], io["wo_a"][:, h2_],
                                eng=nc.sync)
                    elif i >= 8:
                        for pjx in opch.get(i, ()):
                            oproj_chain(pjx, 0)
                        if i < 12:
                            gu_fetch(i - 8)
                # drain: half-1 out-proj chains overlap the last two pairs'
                # softmax tails -- contraction slabs 0..5 (ready) run first,
                # slabs 6..7 wait only on the final pair's at8
                consume_pair(*pend.pop(0))
                stp_sc.close()  # scores PSUM freed (WARs tracked per bank)
                po1p = att_sc.enter_context(
                    tc.tile_pool(name="po1p", bufs=1, space="PSUM"))
                # two chains share each bank and their starts interleave, so
                # a start=True would wipe the sibling's partials (PSUM
                # auto-zero is bank-granular): memset + accumulate-only
                p1t = [po1p.tile([P, 2, 256], f32, name=f"p1_{j}",
                                 tag=f"p1_{j}") for j in range(4)]
                for j in range(4):
                    nc.vector.memset(p1t[j][:], 0.0)
                qs1 = slice(256, 512)
                for po in range(0, 6, 2):
                    if po >= 2 and pend:
                        consume_pair(*pend.pop(0))
                    for pjx in range(ND):
                        nc.tensor.matmul(p1t[pjx // 2][:, pjx % 2, :],
                                         wo_at[pjx][:, po:po + 2, :],
                                         at8T[:, po:po + 2, qs1],
                                         start=False, stop=False,
                                         perf_mode=DR,
                                         skip_group_check=True)
                while pend:
                    consume_pair(*pend.pop(0))
                msp5 = att_sc.enter_context(
                    tc.tile_pool(name="msp5", bufs=1, space="PSUM"))
                ms5 = msp5.tile([1, CHUNK], f32, name="ms5")
                for pjx in range(ND):
                    nc.tensor.matmul(p1t[pjx // 2][:, pjx % 2, :],
                                     wo_at[pjx][:, 6:8, :],
                                     at8T[:, 6:8, qs1],
                                     start=False, stop=True, perf_mode=DR,
                                     skip_group_check=True)
                    if pjx % 2 == 0:
                        continue
                    nc.vector.tensor_add(xF[:, pjx - 1:pjx + 1, qs1],
                                         p1t[pjx // 2][:],
                                         xF[:, pjx - 1:pjx + 1, qs1])
                    # rms2 square + reduce as each s_o*x2 row-pair lands
                    sq = sqp5.tile([P, 2, CHUNK], f32, name="sq5")
                    nc.scalar.activation(sq[:], xF[:, pjx - 1:pjx + 1, :],
                                         AF.Square)
                    for w in range(2):
                        nc.tensor.matmul(ms5[:], onescol_f[:], sq[:, w, :],
                                         start=(pjx == 1 and w == 0),
                                         stop=(pjx == ND - 1 and w == 1))

                sr5 = r2p.tile([1, CHUNK], f32, name="sr5")
                nc.scalar.activation(sr5[:], ms5[:], AF.Sqrt,
                                     bias=eps2b[0:1, :], scale=1.0 / D)
                nc.vector.reciprocal(rro2[:], sr5[:])
                att_sc.close()  # frees the attention + drain PSUM banks

            tap("dbg_at8", at8T[:])

        # xF = s_o*x2 from here on.
        if "dbg_x2" in io:
            nc.sync.dma_start(out=io["dbg_x2"], in_=xF[:])

        # ---- phase 5: rmsnorm2 broadcast + swiglu ffn (bf16) -------------
        with ExitStack() as fctx:
            h2T = fctx.enter_context(tc.tile_pool(name="h2Tp", bufs=1)).tile(
                [P, ND, CHUNK], bf, name="h2T")
            prod = fctx.enter_context(tc.tile_pool(name="prodp", bufs=1)).tile(
                [P, NF, CHUNK], bf, name="prod")

            with ExitStack() as pctx:
                rbp = pctx.enter_context(
                    tc.tile_pool(name="rbp5", bufs=1, space="PSUM"))
                rowp = pctx.enter_context(tc.tile_pool(name="rowp5", bufs=1))

                rb = rbp.tile([P, CHUNK], f32, name="rb5")
                # consts row 1 = 1.0 (h2 must land unscaled for bf16 ffn)
                nc.tensor.matmul(rb[:], consts[0:1, 1, :], rro2[:],
                                 start=True, stop=True)
                # duplicated so h2 muls cover two a-tiles per op; DVE/Pool
                rbc = rowp.tile([P, 2, CHUNK], bf, name="rbc5")
                nc.vector.tensor_copy(rbc[:, 0, :], rb[:])
                nc.scalar.copy(rbc[:, 1, :], rb[:])
                for a in range(0, ND, 2):
                    eng = nc.vector if (a // 2) % 2 == 0 else nc.gpsimd
                    eng.tensor_mul(h2T[:, a:a + 2, :], xF[:, a:a + 2, :],
                                   rbc[:])

            tap("dbg_h2", h2T[:])

            # bf16 copy of s_o*x2 for the PE-side residual accumulation
            # (emitted mid-g/u: only the final iden matmuls consume it)
            xFb = fctx.enter_context(tc.tile_pool(name="xFbp", bufs=1)).tile(
                [P, ND, CHUNK], bf, name="xFb")

            # gate/up products: fo-pairs share PSUM tiles -> one silu + one
            # prod mult per pair, prod mults alternating DVE/Pool
            with ExitStack() as pctx:
                pg = pctx.enter_context(
                    tc.tile_pool(name="pg", bufs=2, space="PSUM"))
                pu = pctx.enter_context(
                    tc.tile_pool(name="pu", bufs=2, space="PSUM"))
                sgp = pctx.enter_context(tc.tile_pool(name="sgp", bufs=3))

                WOORD = [*range(2, NF), 0, 1]
                for fo in range(0, NF, 2):
                    if 8 <= fo < 16:
                        a = (fo - 8) // 2 * 2
                        eng = nc.vector if (a // 2) % 2 == 0 else nc.gpsimd
                        eng.tensor_copy(xFb[:, a:a + 2, :], xF[:, a:a + 2, :])
                    for w in range(2):
                        if fo + w >= FPRE:
                            gu_fetch(fo + w)
                        if fo + w >= NF - 4:
                            wo_fetch(WOORD[fo + w - (NF - 4)])
                    gps = pg.tile([P, 2, CHUNK], f32, name="gps")
                    for w in range(2):
                        for pi in range(ND):
                            nc.tensor.matmul(gps[:, w, :],
                                             wg_tiles[fo + w][:, pi, :],
                                             h2T[:, pi, :],
                                             start=(pi == 0),
                                             stop=(pi == ND - 1))
                    sg = sgp.tile([P, 2, CHUNK], bf, name="sg")
                    nc.scalar.activation(sg[:], gps[:], AF.Silu)
                    ups = pu.tile([P, 2, CHUNK], f32, name="ups")
                    for w in range(2):
                        for pi in range(ND):
                            nc.tensor.matmul(ups[:, w, :],
                                             wu_tiles[fo + w][:, pi, :],
                                             h2T[:, pi, :],
                                             start=(pi == 0),
                                             stop=(pi == ND - 1))
                    nc.vector.tensor_mul(prod[:, fo:fo + 2, :], sg[:], ups[:])

            # wo: out accumulation over all fo
            with ExitStack() as pctx:
                pz = pctx.enter_context(
                    tc.tile_pool(name="pz", bufs=1, space="PSUM"))
                outp = pctx.enter_context(tc.tile_pool(name="outp", bufs=1))

                z2 = [pz.tile([P, CHUNK], f32, name=f"z2_{j}", tag=f"z2_{j}")
                      for j in range(ND)]
                ot = outp.tile([P, ND, CHUNK], bf, name="ot")
                # fo order rotated so the final rounds use early-produced
                # prod tiles (the last prod mults are still in flight)
                outT_p = io["outT"].rearrange("(a p) t -> p a t", p=P)
                for r, fo in enumerate(WOORD[:NF - 4]):
                    if r + 4 < NF:
                        wo_fetch(WOORD[r + 4])
                    wof = wo_tiles[fo]
                    for j in range(ND):
                        nc.tensor.matmul(z2[j][:], wof[:, j * P:(j + 1) * P],
                                         prod[:, fo, :],
                                         start=(r == 0), stop=False)
                # last 4 rounds j-major: early columns close + copy + DMA
                # while later columns still accumulate (shrinks the tail)
                for j in range(ND):
                    for fo in WOORD[NF - 4:]:
                        nc.tensor.matmul(z2[j][:],
                                         wo_tiles[fo][:, j * P:(j + 1) * P],
                                         prod[:, fo, :],
                                         start=False, stop=False)
                    # residual via (1/s_o)-diagonal matmul closes the group
                    nc.tensor.matmul(z2[j][:], iden[:], xFb[:, j, :],
                                     start=False, stop=True)
                    if j % 2 == 0:
                        nc.vector.tensor_copy(ot[:, j, :], z2[j][:])
                    else:
                        nc.scalar.copy(ot[:, j, :], z2[j][:])
                    for hh in range(2):
                        # early j may use the slower SWDGE (hidden under the
                        # remaining matmuls); the last columns stay on SP
                        eng = nc.gpsimd if (hh == 1 and j < 6) else nc.sync
                        dma(outT_p[:, j:j + 1, hh * 256:(hh + 1) * 256],
                            ot[:, j, hh * 256:(hh + 1) * 256], eng=eng)


# --------------------------------------------------------------------------
# host-side sharding / unsharding
# --------------------------------------------------------------------------

def _bf16(x):
    import ml_dtypes
    return np.ascontiguousarray(x.astype(ml_dtypes.bfloat16))


def _f8(x):
    import ml_dtypes
    return np.ascontiguousarray(x.astype(ml_dtypes.float8_e4m3))


def _pow2floor(v):
    return 2.0 ** np.floor(np.log2(v))


def _tile_kmajor(w):
    """[D_in, D_out] -> [P, D_in//P (po-tiles of 128 out-cols), ...] layout
    [p, po, a, o] where w[a*P+p, po*P+o]."""
    din, dout = w.shape
    a, po = din // P, dout // P
    return np.ascontiguousarray(
        w.reshape(a, P, po, P).transpose(1, 2, 0, 3))


def _build_trimask(chunk_start, neg_bias):
    """Violation-bias consts in [s_partition, qhalf, ktile, r] layout.
    The on-device matmul computes st[p, r] += (1/s_o) * trimask[p, qh, j, r];
    trimask carries -s_o*C at (band violation | invalid key), 0 elsewhere."""
    m = np.zeros((2, 4, P, 256), np.float32)
    for qh in range(2):
        c = (np.arange(4 * P)[:, None])            # window key coord [0, 512)
        rr = np.arange(256)[None, :]
        viol = ~((c >= rr + 1) & (c <= rr + WIN))
        invalid = (chunk_start - 256 + qh * 256 + c) < 0
        m[qh] = (viol | invalid).astype(np.float32).reshape(4, P, 256)
    m = m * neg_bias
    return np.ascontiguousarray(m.transpose(2, 0, 1, 3))  # [P, 2, 4, 256]


def compute_scales(qkv_w, gate_w, out_w, ln1_w):
    """Pow2 fold constants from the weights (host side, exact)."""
    tot = NH * DH
    wq_e = (qkv_w[0 * tot:1 * tot] * ln1_w[None, :]).T  # [D, tot]
    wk_e = (qkv_w[1 * tot:2 * tot] * ln1_w[None, :]).T
    wv_e = (qkv_w[2 * tot:3 * tot] * ln1_w[None, :]).T
    wgate_e = (gate_w * ln1_w[None, :]).T
    wout_e = out_w.T                                    # [tot, D]

    m_q = np.abs(wq_e).max(0) + 1e-30                   # per out-channel d
    m_k = np.abs(wk_e).max(0) + 1e-30
    c_glob = _pow2floor((TGT * TGT / (m_q * m_k)).min())
    s_d = _pow2floor(np.sqrt(c_glob * m_k / m_q))
    s_v = _pow2floor(TGT / np.abs(wv_e).max())
    s_g = _pow2floor(TGT / np.abs(wgate_e).max())
    s_o = _pow2floor(TGT / np.abs(wout_e).max())
    return {
        "c_glob": float(c_glob), "s_d": s_d, "s_v": float(s_v),
        "s_g": float(s_g), "s_o": float(s_o),
        "EXPS": float(SCALE / (SA * SA * c_glob)),
        "SIG": float(1.0 / (SA * s_g)),
        "KV": float(SV8 / (SA * s_v)),
        "EPS2": float(EPS * s_o * s_o),
    }


def make_in_maps(x, ln1_w, qkv_w, gate_w, out_w, ln2_w, wg, wu, wo, sc):
    tot = NH * DH
    # fold rmsnorm weights into the consuming projection weights
    wq_e = (qkv_w[0 * tot:1 * tot] * ln1_w[None, :]).T  # [D(in), D(out)]
    wk_e = (qkv_w[1 * tot:2 * tot] * ln1_w[None, :]).T
    wv_e = (qkv_w[2 * tot:3 * tot] * ln1_w[None, :]).T
    wgate_e = (gate_w * ln1_w[None, :]).T
    wout_e = out_w.T                                    # [tot, D]
    wg_e = (wg * ln2_w[None, :]).T                      # [D, DFF]
    wu_e = (wu * ln2_w[None, :]).T
    wo_e = wo.T                                         # [DFF, D]

    # fp8 quantization with folded scales
    wq8 = wq_e * sc["s_d"][None, :]
    wk8 = wk_e * (sc["c_glob"] / sc["s_d"])[None, :]
    wv8 = wv_e * sc["s_v"]
    wgt8 = wgate_e * sc["s_g"]
    wout8 = wout_e * sc["s_o"]

    # pre-tiled device layouts
    wv_l = _tile_kmajor(wv8).reshape(P, 4, 2, ND, P).transpose(
        0, 1, 3, 2, 4).reshape(P, 4, ND, 256)  # [p, ng, a, 256]
    wg_l = _tile_kmajor(wg_e)                            # [p, fo, a, o]
    wu_l = _tile_kmajor(wu_e)
    wo_l = np.ascontiguousarray(
        wo_e.reshape(NF, P, D).transpose(1, 0, 2))       # [p, fo, d]

    # both rows 1.0: rms1's dequant is deferred (raw fp8 x feeds the
    # projections), rms2's h2 lands unscaled for the bf16 ffn
    consts = np.ones((1, 2, P), np.float32)

    shared = {
        "wq": _f8(_tile_kmajor(wq8)),
        "wk": _f8(_tile_kmajor(wk8)),
        "wv": _f8(wv_l),
        "wgt": _f8(_tile_kmajor(wgt8)),
        "wo_a": _f8(_tile_kmajor(wout8)),
        "wg": _bf16(wg_l),
        "wu": _bf16(wu_l),
        "wo": _bf16(wo_l),
        "consts": _bf16(consts),
        "iden": _bf16(np.eye(P, dtype=np.float32) / sc["s_o"]),
    }

    in_maps = []
    for c in range(NCORES):
        b, ck = divmod(c, T // CHUNK)
        cs = ck * CHUNK
        xw = np.zeros((LT, D), np.float32)
        lo = cs - HALO
        xw[max(0, -lo):] = x[b, max(lo, 0):cs + CHUNK]
        m = dict(shared)
        xt = np.ascontiguousarray(xw.T)                  # [D, LT]
        m["xb"] = _f8((xt * SA).reshape(ND, P, LT).transpose(1, 0, 2))
        m["xf"] = np.ascontiguousarray(
            (xt[:, HALO:] * sc["s_o"]).reshape(ND, P, CHUNK).transpose(1, 0, 2))
        m["trimask"] = _bf16(_build_trimask(cs, -30.0 / sc["EXPS"] * sc["s_o"]))
        in_maps.append(m)
    return in_maps


def gather_output(results):
    out = np.empty((B, T, D), np.float32)
    for c in range(NCORES):
        b, ck = divmod(c, T // CHUNK)
        out[b, ck * CHUNK:(ck + 1) * CHUNK] = results[c]["outT"].T
    return out


def kernel(**inputs):
    from concourse.bass_utils import run_bass_kernel_spmd

    sc = compute_scales(inputs["qkv_w"], inputs["gate_w"], inputs["out_w"],
                        inputs["ln1_w"])
    if "nc" not in _CACHE:
        _CACHE["nc"] = build_program(sc)
        _CACHE["sc"] = sc
    nc = _CACHE["nc"]

    in_maps = make_in_maps(**inputs, sc=sc)
    res = run_bass_kernel_spmd(nc, in_maps, core_ids=list(range(NCORES)))
    return gather_output(res.results)


if __name__ == "__main__":
    rng = np.random.default_rng(0)
    ins = {
        "x": rng.standard_normal((B, T, D), dtype=np.float32),
        "ln1_w": np.ones(D, np.float32),
        "qkv_w": rng.standard_normal((3 * NH * DH, D), dtype=np.float32) * 0.02,
        "gate_w": rng.standard_normal((NH * DH, D), dtype=np.float32) * 0.04,
        "out_w": rng.standard_normal((D, NH * DH), dtype=np.float32) * 0.04,
        "ln2_w": np.ones(D, np.float32),
        "wg": rng.standard_normal((DFF, D), dtype=np.float32) * 0.02,
        "wu": rng.standard_normal((DFF, D), dtype=np.float32) * 0.02,
        "wo": rng.standard_normal((D, DFF), dtype=np.float32) * 0.02,
    }
    out = kernel(**ins)
    print("out", out.shape, out.dtype, float(np.abs(out).mean()))
